# revision 1
# baseline (speedup 1.0000x reference)
"""Gated-attention (Qwen-style) Trainium2 kernel.

Sharding (8 cores): data-parallel over batch (2) x tensor-parallel over head
groups (4). Core c handles batch b=c//4 and head group g=c%4: q heads
4g..4g+3, kv heads 2g..2g+1, gate logits 4g..4g+3, w_o columns 512g..512g+512.
Each core computes a partial output y_g = attn_out_g @ w_o[:, cols_g].T; the
host sums the 4 partials per batch.

Device-side layout ("layout A"): qkv projection computed with head-rows on the
M axis, so qT/kT come out as [d, tokens] (ready for the QK^T matmul) and v is
PE-transposed to [tokens, d] (ready as attn@v stationary operand). Softmax is
computed in transposed [k, q] layout: exp on ACT straight from PSUM, the
denominator via a ones-column matmul, normalization + sigmoid(gate) applied as
a per-token scale broadcast over partitions by gpsimd.

All matmuls use float32r (full fp32 bits in SBUF, relaxed PE precision,
1 cycle/row for moving dim >= 256; plain fp32 is 4x slower).
"""

import os
from contextlib import ExitStack

import numpy as np

B, S, HID = 2, 2048, 2048
NH, NKV, HD = 16, 8, 128
GATE = NH
KV_DIM = NKV * HD

N_CORES = 8
TPG = 4            # tensor-parallel group size (head groups)
QH = NH // TPG     # q heads per core = 4
KVH = NKV // TPG   # kv heads per core = 2
IB = 256           # phase-1 token block
NB = S // IB       # 8 blocks
JT = S // 128      # 16 key tiles
IBLK = 512         # phase-2 query block
NI = S // IBLK     # 4 query blocks
SCALE = 1.0 / float(np.sqrt(HD))

_CACHE = {}

LAST_EXEC_NS = None
LAST_RESULTS = None


def _build_program():
    import concourse.bass as bass
    import concourse.mybir as mybir
    from concourse import bacc
    from concourse.tile import TileContext

    F32 = mybir.dt.float32
    F32R = mybir.dt.float32r
    AF = mybir.ActivationFunctionType

    nc = bacc.Bacc()

    xT_d = nc.dram_tensor("xT", [HID, S], F32R, kind="ExternalInput")
    wqkvT_d = nc.dram_tensor("wqkvT", [HID, 1024], F32R, kind="ExternalInput")
    wgT_d = nc.dram_tensor("wgT", [HID, QH], F32R, kind="ExternalInput")
    woT_d = nc.dram_tensor("woT", [QH * HD, HID], F32R, kind="ExternalInput")
    cosT_d = nc.dram_tensor("cosT", [HD, S], F32, kind="ExternalInput")
    sinT_d = nc.dram_tensor("sinT", [HD, S], F32, kind="ExternalInput")
    rotm_d = nc.dram_tensor("rotm", [HD, HD], F32R, kind="ExternalInput")
    ident_d = nc.dram_tensor("ident", [128, 128], F32, kind="ExternalInput")
    ones_d = nc.dram_tensor("ones", [128, 1], F32R, kind="ExternalInput")
    y_d = nc.dram_tensor("y", [S, HID], F32, kind="ExternalOutput")

    with TileContext(nc) as tc, ExitStack() as persist:
        const = persist.enter_context(tc.tile_pool(name="const", bufs=1))
        rotm_sb = const.tile([HD, HD], F32R, tag="rotm", name="rotm_sb")
        nc.sync.dma_start(out=rotm_sb, in_=rotm_d[:, :])
        ident_sb = const.tile([128, 128], F32, tag="ident", name="ident_sb")
        nc.sync.dma_start(out=ident_sb, in_=ident_d[:, :])
        ones_sb = const.tile([128, 1], F32R, tag="ones", name="ones_sb")
        nc.sync.dma_start(out=ones_sb, in_=ones_d[:, :])

        qk_pool = persist.enter_context(tc.tile_pool(name="qk", bufs=1))
        qk_sb = [qk_pool.tile([128, S], F32R, tag=f"qk{r}", name=f"qk{r}") for r in range(QH + KVH)]
        v_pool = persist.enter_context(tc.tile_pool(name="v", bufs=1))
        v_sb = [v_pool.tile([128, KVH * HD], F32R, tag=f"v{t}", name=f"v{t}") for t in range(JT)]
        g_pool = persist.enter_context(tc.tile_pool(name="g", bufs=1))
        gT_sb = g_pool.tile([QH, S], F32, tag="gT", name="gT")
        sgT_sb = g_pool.tile([QH, S], F32, tag="sgT", name="sgT")

        # ---------------- phase 1: qkv projection + rope + v transpose -----
        with ExitStack() as ph1:
            wpool = ph1.enter_context(tc.tile_pool(name="w", bufs=1))
            wsb = [wpool.tile([128, 1024], F32R, tag=f"w{h}", name=f"w{h}") for h in range(16)]
            wg_sb = [wpool.tile([128, QH], F32R, tag=f"wg{h}", name=f"wg{h}") for h in range(16)]
            for h in range(16):
                nc.sync.dma_start(out=wsb[h], in_=wqkvT_d[128 * h:128 * (h + 1), :])
                nc.sync.dma_start(out=wg_sb[h], in_=wgT_d[128 * h:128 * (h + 1), :])

            xpool = ph1.enter_context(tc.tile_pool(name="x", bufs=17))
            cspool = ph1.enter_context(tc.tile_pool(name="cs", bufs=2))
            tmppool = ph1.enter_context(tc.tile_pool(name="tmp", bufs=2))
            vrawpool = ph1.enter_context(tc.tile_pool(name="vraw", bufs=2))
            eTpool = ph1.enter_context(tc.tile_pool(name="eT", bufs=1))

            ps_acc = ph1.enter_context(tc.tile_pool(name="acc", bufs=4, space="PSUM"))
            ps_rot = ph1.enter_context(tc.tile_pool(name="rot", bufs=1, space="PSUM"))
            ps_tp = ph1.enter_context(tc.tile_pool(name="tp", bufs=2, space="PSUM"))
            ps_g = ph1.enter_context(tc.tile_pool(name="psg", bufs=1, space="PSUM"))

            for ib in range(NB):
                sl = slice(IB * ib, IB * (ib + 1))
                xb = []
                for h in range(16):
                    xt = xpool.tile([128, IB], F32R, tag="x", name="x")
                    nc.sync.dma_start(out=xt, in_=xT_d[128 * h:128 * (h + 1), sl])
                    xb.append(xt)
                cs = cspool.tile([HD, IB], F32, tag="cs", name="cs")
                nc.sync.dma_start(out=cs, in_=cosT_d[:, sl])
                sn = cspool.tile([HD, IB], F32, tag="sn", name="sn")
                nc.sync.dma_start(out=sn, in_=sinT_d[:, sl])

                # gate logits first (so x tiles' last reader is the rg loop)
                psg = ps_g.tile([QH, IB], F32, tag="psg", name="psg")
                for h in range(16):
                    nc.tensor.matmul(psg, wg_sb[h], xb[h],
                                     start=(h == 0), stop=(h == 15))
                nc.vector.tensor_copy(gT_sb[:, sl], psg)

                for rg in range(2):
                    accs = [ps_acc.tile([128, IB], F32, tag="acc", name="acc") for _ in range(4)]
                    for h in range(16):
                        for r4 in range(4):
                            r = 4 * rg + r4
                            nc.tensor.matmul(
                                accs[r4], wsb[h][:, 128 * r:128 * (r + 1)], xb[h],
                                start=(h == 0), stop=(h == 15))
                    for r4 in range(4):
                        r = 4 * rg + r4
                        if r < QH + KVH:  # q or k row-tile: apply rope
                            craw = tmppool.tile([128, IB], F32R, tag="craw", name="craw")
                            nc.vector.tensor_copy(craw, accs[r4])
                            rps = ps_rot.tile([128, IB], F32, tag="rot", name="rot")
                            nc.tensor.matmul(rps, rotm_sb, craw, start=True, stop=True)
                            t1 = tmppool.tile([128, IB], F32R, tag="t1", name="t1")
                            nc.vector.tensor_mul(t1, accs[r4], cs)
                            t2 = tmppool.tile([128, IB], F32R, tag="t2", name="t2")
                            nc.vector.tensor_mul(t2, rps, sn)
                            nc.vector.tensor_add(qk_sb[r][:, sl], t1, t2)
                        else:  # v row-tile: transpose to [tokens, d]
                            vraw = vrawpool.tile([128, IB], F32, tag="vraw", name="vraw")
                            nc.vector.tensor_copy(vraw, accs[r4])
                            vh = r - (QH + KVH)
                            for s2 in range(IB // 128):
                                tp = ps_tp.tile([128, 128], F32, tag="tp", name="tp")
                                nc.tensor.transpose(
                                    tp, vraw[:, 128 * s2:128 * (s2 + 1)], ident_sb)
                                tt = (IB // 128) * ib + s2
                                nc.vector.tensor_copy(
                                    v_sb[tt][:, 128 * vh:128 * (vh + 1)], tp)

            # sigmoid(gate) = 1 / (1 + exp(-g)), rows stay on partitions 0-3
            eT = eTpool.tile([QH, S], F32, tag="eT", name="eT")
            nc.scalar.activation(out=eT, in_=gT_sb, func=AF.Exp, scale=-1.0)
            nc.vector.tensor_scalar_add(eT, eT, 1.0)
            nc.vector.reciprocal(sgT_sb, eT)

        # ---------------- phase 2+3: attention, gate, out-projection -------
        with ExitStack() as ph2:
            wopool = ph2.enter_context(tc.tile_pool(name="wo", bufs=1))
            wo_sb = [wopool.tile([128, HID], F32R, tag=f"wo{i}", name=f"wo{i}") for i in range(4)]
            for cc in range(4):
                nc.sync.dma_start(out=wo_sb[cc], in_=woT_d[128 * cc:128 * (cc + 1), :])
            oc_pool = ph2.enter_context(tc.tile_pool(name="oc", bufs=1))
            OC = [oc_pool.tile([128, S], F32R, tag=f"oc{h}", name=f"oc{h}") for h in range(QH)]
            epool = ph2.enter_context(tc.tile_pool(name="e", bufs=4))
            scpool = ph2.enter_context(tc.tile_pool(name="sc", bufs=2))
            bcpool = ph2.enter_context(tc.tile_pool(name="bc", bufs=2))
            ypool = ph2.enter_context(tc.tile_pool(name="y", bufs=4))
            sgrow = ph2.enter_context(tc.tile_pool(name="sgr", bufs=2))

            ps_s = ph2.enter_context(tc.tile_pool(name="pss", bufs=2, space="PSUM"))
            ps_o = ph2.enter_context(tc.tile_pool(name="pso", bufs=2, space="PSUM"))
            ps_sum = ph2.enter_context(tc.tile_pool(name="psum", bufs=2, space="PSUM"))
            ps_y = ph2.enter_context(tc.tile_pool(name="psy", bufs=2, space="PSUM"))

            for i in range(NI):
                isl = slice(IBLK * i, IBLK * (i + 1))
                for h in range(QH):
                    kv = h // 2
                    pso = ps_o.tile([128, IBLK], F32, tag="pso", name="pso")
                    pssum = ps_sum.tile([1, IBLK], F32, tag="psum", name="psums")
                    for j in range(JT):
                        pss = ps_s.tile([128, IBLK], F32, tag="pss", name="pss")
                        nc.tensor.matmul(
                            pss, qk_sb[QH + kv][:, 128 * j:128 * (j + 1)],
                            qk_sb[h][:, isl], start=True, stop=True)
                        e = epool.tile([128, IBLK], F32R, tag="e", name="e")
                        nc.scalar.activation(out=e, in_=pss, func=AF.Exp, scale=SCALE)
                        nc.tensor.matmul(pso, v_sb[j][:, 128 * kv:128 * (kv + 1)], e,
                                         start=(j == 0), stop=(j == JT - 1))
                        nc.tensor.matmul(pssum, ones_sb, e,
                                         start=(j == 0), stop=(j == JT - 1))
                    # scale = sigmoid(gate) / sums, per token of this i-block
                    sgr = sgrow.tile([1, IBLK], F32, tag="sgr", name="sgr")
                    nc.sync.dma_start(out=sgr, in_=sgT_sb[h:h + 1, isl])
                    rec = scpool.tile([1, IBLK], F32, tag="rec", name="rec")
                    nc.vector.reciprocal(rec, pssum)
                    sc = scpool.tile([1, IBLK], F32, tag="sc", name="sc")
                    nc.vector.tensor_mul(sc, rec, sgr)
                    bc = bcpool.tile([128, IBLK], F32, tag="bc", name="bc")
                    nc.gpsimd.partition_broadcast(bc, sc, channels=128)
                    nc.vector.tensor_mul(OC[h][:, isl], pso, bc)
                # out-projection for this i-block's token tiles
                for t in range(4 * i, 4 * i + 4):
                    for o in range(4):
                        psy = ps_y.tile([128, IBLK], F32, tag="psy", name="psy")
                        for cc in range(4):
                            nc.tensor.matmul(
                                psy, OC[cc][:, 128 * t:128 * (t + 1)],
                                wo_sb[cc][:, IBLK * o:IBLK * (o + 1)],
                                start=(cc == 0), stop=(cc == 3))
                        ysb = ypool.tile([128, IBLK], F32, tag="y", name="ysb")
                        nc.vector.tensor_copy(ysb, psy)
                        nc.sync.dma_start(
                            out=y_d[128 * t:128 * (t + 1), IBLK * o:IBLK * (o + 1)],
                            in_=ysb)

    nc.finalize()
    return nc


def kernel(hidden_states, cos, sin, w_qkv, w_o):
    global LAST_EXEC_NS, LAST_RESULTS
    from concourse.bass_utils import run_bass_kernel_spmd

    hidden_states = np.asarray(hidden_states, dtype=np.float32)
    cos = np.asarray(cos, dtype=np.float32)
    sin = np.asarray(sin, dtype=np.float32)
    w_qkv = np.asarray(w_qkv, dtype=np.float32)
    w_o = np.asarray(w_o, dtype=np.float32)

    if "nc" not in _CACHE:
        _CACHE["nc"] = _build_program()
    nc = _CACHE["nc"]

    cosT = np.ascontiguousarray(cos.T)
    sinT = np.ascontiguousarray(sin.T)
    rotm = np.zeros((HD, HD), dtype=np.float32)
    for i in range(HD // 2):
        rotm[i + HD // 2, i] = -1.0   # rot[d'] = -q[d'+64] for d' < 64
        rotm[i, i + HD // 2] = 1.0    # rot[d'] = +q[d'-64] for d' >= 64
    ident = np.eye(128, dtype=np.float32)
    ones = np.ones((128, 1), dtype=np.float32)

    xT = [np.ascontiguousarray(hidden_states[b].T) for b in range(B)]
    in_maps = []
    for c in range(N_CORES):
        b, g = divmod(c, TPG)
        qr = w_qkv[512 * g:512 * (g + 1)]
        kr = w_qkv[HID + GATE + 256 * g:HID + GATE + 256 * (g + 1)]
        vr = w_qkv[HID + GATE + KV_DIM + 256 * g:HID + GATE + KV_DIM + 256 * (g + 1)]
        wqkvT = np.ascontiguousarray(np.concatenate([qr, kr, vr], axis=0).T)
        wgT = np.ascontiguousarray(w_qkv[HID + QH * g:HID + QH * (g + 1)].T)
        woT = np.ascontiguousarray(w_o[:, 512 * g:512 * (g + 1)].T)
        in_maps.append({
            "xT": xT[b], "wqkvT": wqkvT, "wgT": wgT, "woT": woT,
            "cosT": cosT, "sinT": sinT, "rotm": rotm, "ident": ident,
            "ones": ones,
        })

    trace = bool(int(os.environ.get("KERNEL_TRACE", "0")))
    out = run_bass_kernel_spmd(nc, in_maps, list(range(N_CORES)), trace=trace)
    LAST_EXEC_NS = out.exec_time_ns
    LAST_RESULTS = out
    y = np.zeros((B, S, HID), dtype=np.float32)
    for c in range(N_CORES):
        b = c // TPG
        y[b] += out.results[c]["y"]
    return y



# revision 6
# speedup vs baseline: 1.4705x; 1.4705x over previous
"""Gated-attention (Qwen-style) Trainium2 kernel, v2 (bf16).

Sharding (8 cores): data-parallel over batch (2) x tensor-parallel over head
groups (4). Core c handles batch b=c//4 and head group g=c%4: q heads
4g..4g+3, kv heads 2g..2g+1, gate logits 4g..4g+3, w_o columns 512g..512g+512.
Each core computes a partial output y_g = attn_out_g @ w_o[:, cols_g].T in
bf16; the host sums the 4 partials per batch in fp32.

v2 changes vs v1:
- bf16 operands everywhere on the PE (fp32 PSUM accumulation): enables fast
  weight load (halves LDWEIGHTS), halves SBUF footprint and HBM traffic.
- V projected directly to [tokens, d] layout with x-chunk-stationary matmuls
  (no PE transposes, no extra evictions).
- exp batched over two PSUM banks per ACT instruction ([128,1024]) and double
  buffered so ACT hides fully behind PE.
- softmax denominators for all 4 heads of a q-block accumulate into one PSUM
  tile [4, 512] via per-head one-hot ones columns; normalization uses
  reciprocal_approx_fast (no 8-cyc/elem RECIPROCALs, no extra table loads).
- sigmoid(gate)/denominator folded into one broadcast scale per q-block:
  4 tiny SBUF DMAs -> [1, 2048] -> one gpsimd partition_broadcast.
- x/weights loaded with full-row 1MB-class DMAs up front; y written as
  [128, 2048] bf16 tiles (host upcasts and reduces).
"""

import os
from contextlib import ExitStack

import numpy as np

B, S, HID = 2, 2048, 2048
NH, NKV, HD = 16, 8, 128
GATE = NH
KV_DIM = NKV * HD

N_CORES = 8
TPG = 4            # tensor-parallel group size (head groups)
QH = NH // TPG     # q heads per core = 4
KVH = NKV // TPG   # kv heads per core = 2
IB = 512           # phase-1 token block
NB = S // IB       # 4 blocks
JT = S // 128      # 16 key tiles
IBLK = 512         # phase-2 query block
NI = S // IBLK     # 4 query blocks
QKVC = 8 * HD + 4  # 1028 qkv+gate columns per core (q 512, k 256, gate 4, v 256)
SCALE = 1.0 / float(np.sqrt(HD))

_CACHE = {}

LAST_EXEC_NS = None
LAST_RESULTS = None


def _build_program():
    import concourse.bass as bass
    import concourse.mybir as mybir
    from concourse import bacc
    from concourse.tile import TileContext

    F32 = mybir.dt.float32
    BF16 = mybir.dt.bfloat16
    AF = mybir.ActivationFunctionType

    nc = bacc.Bacc()

    xT_d = nc.dram_tensor("xT", [HID, S], BF16, kind="ExternalInput")
    wqkvT_d = nc.dram_tensor("wqkvT", [HID, QKVC], BF16, kind="ExternalInput")
    woT_d = nc.dram_tensor("woT", [QH * HD, HID], BF16, kind="ExternalInput")
    cosT_d = nc.dram_tensor("cosT", [HD, S], BF16, kind="ExternalInput")
    sinT_d = nc.dram_tensor("sinT", [HD, S], BF16, kind="ExternalInput")
    rotm_d = nc.dram_tensor("rotm", [HD, HD], BF16, kind="ExternalInput")
    oneh_d = nc.dram_tensor("oneh", [128, 4 * QH], BF16, kind="ExternalInput")
    y_d = nc.dram_tensor("y", [S, HID], BF16, kind="ExternalOutput")

    with TileContext(nc) as tc, ExitStack() as persist:
        const = persist.enter_context(tc.tile_pool(name="const", bufs=1))
        rotm_sb = const.tile([HD, HD], BF16, tag="rotm", name="rotm_sb")
        nc.sync.dma_start(out=rotm_sb, in_=rotm_d[:, :])
        oneh_sb = const.tile([128, 4 * QH], BF16, tag="oneh", name="oneh_sb")
        nc.sync.dma_start(out=oneh_sb, in_=oneh_d[:, :])

        qk_pool = persist.enter_context(tc.tile_pool(name="qk", bufs=1))
        qk_sb = [qk_pool.tile([128, S], BF16, tag=f"qk{r}", name=f"qk{r}")
                 for r in range(QH + KVH)]
        v_pool = persist.enter_context(tc.tile_pool(name="v", bufs=1))
        v_sb = [v_pool.tile([128, KVH * HD], BF16, tag=f"v{t}", name=f"v{t}")
                for t in range(JT)]
        g_pool = persist.enter_context(tc.tile_pool(name="g", bufs=1))
        sg4 = g_pool.tile([QH, S], F32, tag="sg4", name="sg4")

        # ---------------- phase 1: qkv projection + rope + direct-v ---------
        with ExitStack() as ph1:
            wpool = ph1.enter_context(tc.tile_pool(name="w", bufs=1))
            wsb = [wpool.tile([128, QKVC], BF16, tag=f"w{h}", name=f"w{h}")
                   for h in range(16)]
            xpool = ph1.enter_context(tc.tile_pool(name="x", bufs=1))
            xb = [xpool.tile([128, S], BF16, tag=f"x{h}", name=f"x{h}")
                  for h in range(16)]
            for h in range(16):
                nc.sync.dma_start(out=xb[h], in_=xT_d[128 * h:128 * (h + 1), :])
                nc.sync.dma_start(out=wsb[h], in_=wqkvT_d[128 * h:128 * (h + 1), :])
            cspool = ph1.enter_context(tc.tile_pool(name="cs", bufs=1))
            cs_sb = cspool.tile([HD, S], BF16, tag="cs", name="cs_sb")
            nc.sync.dma_start(out=cs_sb, in_=cosT_d[:, :])
            sn_sb = cspool.tile([HD, S], BF16, tag="sn", name="sn_sb")
            nc.sync.dma_start(out=sn_sb, in_=sinT_d[:, :])

            gpool = ph1.enter_context(tc.tile_pool(name="gf", bufs=1))
            gf32 = gpool.tile([QH, S], F32, tag="gf32", name="gf32")
            tmppool = ph1.enter_context(tc.tile_pool(name="tmp", bufs=3))

            ps_acc = ph1.enter_context(tc.tile_pool(name="acc", bufs=3, space="PSUM"))
            ps_rot = ph1.enter_context(tc.tile_pool(name="rot", bufs=2, space="PSUM"))
            ps_v = ph1.enter_context(tc.tile_pool(name="psv", bufs=2, space="PSUM"))
            ps_g = ph1.enter_context(tc.tile_pool(name="psg", bufs=1, space="PSUM"))

            for ib in range(NB):
                sl = slice(IB * ib, IB * (ib + 1))
                # q/k row-tiles with rope
                for r in range(QH + KVH):
                    acc = ps_acc.tile([128, IB], F32, tag="acc", name="acc")
                    for h in range(16):
                        nc.tensor.matmul(acc, wsb[h][:, 128 * r:128 * (r + 1)],
                                         xb[h][:, sl], start=(h == 0), stop=(h == 15))
                    craw = tmppool.tile([128, IB], BF16, tag="craw", name="craw")
                    nc.vector.tensor_copy(craw, acc)
                    rps = ps_rot.tile([128, IB], F32, tag="rot", name="rot")
                    nc.tensor.matmul(rps, rotm_sb, craw, start=True, stop=True)
                    t1 = tmppool.tile([128, IB], BF16, tag="t1", name="t1")
                    nc.vector.tensor_mul(t1, craw, cs_sb[:, sl])
                    t2 = tmppool.tile([128, IB], BF16, tag="t2", name="t2")
                    nc.vector.tensor_mul(t2, rps, sn_sb[:, sl])
                    nc.vector.tensor_add(qk_sb[r][:, sl], t1, t2)
                # gate logits [4, IB]
                psg = ps_g.tile([QH, IB], F32, tag="psg", name="psg")
                for h in range(16):
                    nc.tensor.matmul(psg, wsb[h][:, 6 * HD:6 * HD + 4],
                                     xb[h][:, sl], start=(h == 0), stop=(h == 15))
                nc.vector.tensor_copy(gf32[:, sl], psg)
                # v directly in [tokens, d]: x-chunk stationary, wv moving
                for t2i in range(IB // 128):
                    tt = (IB // 128) * ib + t2i
                    vps = ps_v.tile([128, KVH * HD], F32, tag="vps", name="vps")
                    for h in range(16):
                        nc.tensor.matmul(
                            vps, xb[h][:, 128 * tt:128 * (tt + 1)],
                            wsb[h][:, 6 * HD + 4:QKVC],
                            start=(h == 0), stop=(h == 15))
                    nc.vector.tensor_copy(v_sb[tt], vps)

            # sigmoid(gate) once: sg4 = 1 / (1 + exp(-g))
            u4 = gpool.tile([QH, S], F32, tag="u4", name="u4")
            nc.scalar.activation(out=u4, in_=gf32, func=AF.Exp, scale=-1.0)
            nc.vector.tensor_scalar_add(u4, u4, 1.0)
            nc.vector.reciprocal_approx_fast(out=sg4, in_=u4)

        # ---------------- phase 2: attention + gate + out-projection --------
        with ExitStack() as ph2:
            wopool = ph2.enter_context(tc.tile_pool(name="wo", bufs=1))
            wo_sb = [wopool.tile([128, HID], BF16, tag=f"wo{i}", name=f"wo{i}")
                     for i in range(QH)]
            for cc in range(QH):
                nc.sync.dma_start(out=wo_sb[cc], in_=woT_d[128 * cc:128 * (cc + 1), :])
            oc_pool = ph2.enter_context(tc.tile_pool(name="oc", bufs=2))
            epool = ph2.enter_context(tc.tile_pool(name="e", bufs=3))
            tr0pool = ph2.enter_context(tc.tile_pool(name="tr0", bufs=10))
            trpool = ph2.enter_context(tc.tile_pool(name="tr", bufs=4))
            scpool = ph2.enter_context(tc.tile_pool(name="sc", bufs=2))
            flatpool = ph2.enter_context(tc.tile_pool(name="fl", bufs=2))
            bcpool = ph2.enter_context(tc.tile_pool(name="bc", bufs=2))
            ypool = ph2.enter_context(tc.tile_pool(name="y", bufs=2))

            ps_s = ph2.enter_context(tc.tile_pool(name="pss", bufs=2, space="PSUM"))
            ps_o = ph2.enter_context(tc.tile_pool(name="pso", bufs=1, space="PSUM"))
            ps_den = ph2.enter_context(tc.tile_pool(name="psden", bufs=1, space="PSUM"))
            ps_y = ph2.enter_context(tc.tile_pool(name="psy", bufs=2, space="PSUM"))

            for i in range(NI):
                isl = slice(IBLK * i, IBLK * (i + 1))
                psden = ps_den.tile([QH, IBLK], F32, tag="psden", name="psden")
                oc_i = []
                for h in range(QH):
                    kv = h // 2
                    pso = ps_o.tile([128, IBLK], F32, tag="pso", name="pso")
                    lvl = []
                    for jp in range(JT // 2):
                        ps2 = ps_s.tile([128, 1024], F32, tag="pss", name="pss")
                        for j2 in range(2):
                            j = 2 * jp + j2
                            nc.tensor.matmul(
                                ps2[:, 512 * j2:512 * (j2 + 1)],
                                qk_sb[QH + kv][:, 128 * j:128 * (j + 1)],
                                qk_sb[h][:, isl], start=True, stop=True)
                        e2 = epool.tile([128, 1024], BF16, tag="e2", name="e2")
                        nc.scalar.activation(out=e2, in_=ps2, func=AF.Exp, scale=SCALE)
                        for j2 in range(2):
                            j = 2 * jp + j2
                            first = (jp == 0 and j2 == 0)
                            last = (jp == JT // 2 - 1 and j2 == 1)
                            nc.tensor.matmul(
                                pso, v_sb[j][:, 128 * kv:128 * (kv + 1)],
                                e2[:, 512 * j2:512 * (j2 + 1)],
                                start=first, stop=last)
                        # denominator partial: fold the two 512-wide k-tiles
                        a0 = tr0pool.tile([128, IBLK], BF16, tag="tr0", name="tr0")
                        nc.vector.tensor_add(a0, e2[:, 0:512], e2[:, 512:1024])
                        lvl.append(a0)
                    # add-tree 8 -> 4 -> 2 -> 1 k-partials on DVE
                    li = 1
                    while len(lvl) > 1:
                        nxt = []
                        for m in range(0, len(lvl), 2):
                            b0 = trpool.tile([128, IBLK], BF16, tag=f"tr{li}",
                                             name=f"tr{li}")
                            nc.vector.tensor_add(b0, lvl[m], lvl[m + 1])
                            nxt.append(b0)
                        lvl = nxt
                        li += 1
                    # single per-head denominator matmul into row h of psden
                    nc.tensor.matmul(psden, oneh_sb[:, 4 * h:4 * (h + 1)], lvl[0],
                                     start=(h == 0), stop=(h == QH - 1))
                    oc = oc_pool.tile([128, IBLK], BF16, tag=f"oc{h}", name=f"oc{h}")
                    nc.vector.tensor_copy(oc, pso)
                    oc_i.append(oc)
                # scale = sigmoid(gate) / denom, flattened + broadcast once
                rden = scpool.tile([QH, IBLK], F32, tag="rden", name="rden")
                nc.vector.reciprocal_approx_fast(out=rden, in_=psden)
                sc4 = scpool.tile([QH, IBLK], F32, tag="sc4", name="sc4")
                nc.vector.tensor_mul(sc4, rden, sg4[:, isl])
                scflat = flatpool.tile([1, QH * IBLK], F32, tag="scflat", name="scflat")
                for h in range(QH):
                    nc.sync.dma_start(out=scflat[0:1, IBLK * h:IBLK * (h + 1)],
                                      in_=sc4[h:h + 1, :])
                bc = bcpool.tile([128, QH * IBLK], F32, tag="bc", name="bc")
                nc.gpsimd.partition_broadcast(bc, scflat, channels=128)
                for h in range(QH):
                    nc.vector.tensor_mul(oc_i[h], oc_i[h],
                                         bc[:, IBLK * h:IBLK * (h + 1)])
                # out-projection for this i-block's token tiles
                for t2i in range(IBLK // 128):
                    t = (IBLK // 128) * i + t2i
                    ysb = ypool.tile([128, HID], BF16, tag="ysb", name="ysb")
                    for o in range(4):
                        psy = ps_y.tile([128, IBLK], F32, tag="psy", name="psy")
                        for cc in range(QH):
                            nc.tensor.matmul(
                                psy, oc_i[cc][:, 128 * t2i:128 * (t2i + 1)],
                                wo_sb[cc][:, IBLK * o:IBLK * (o + 1)],
                                start=(cc == 0), stop=(cc == QH - 1))
                        nc.vector.tensor_copy(ysb[:, IBLK * o:IBLK * (o + 1)], psy)
                    nc.sync.dma_start(out=y_d[128 * t:128 * (t + 1), :], in_=ysb)

    nc.finalize()
    return nc


def kernel(hidden_states, cos, sin, w_qkv, w_o):
    global LAST_EXEC_NS, LAST_RESULTS
    import ml_dtypes
    from concourse.bass_utils import run_bass_kernel_spmd

    BF = ml_dtypes.bfloat16
    hidden_states = np.asarray(hidden_states, dtype=np.float32)
    w_qkv = np.asarray(w_qkv, dtype=np.float32)
    w_o = np.asarray(w_o, dtype=np.float32)

    if "nc" not in _CACHE:
        _CACHE["nc"] = _build_program()
    nc = _CACHE["nc"]

    cosT = np.ascontiguousarray(np.asarray(cos, dtype=np.float32).T).astype(BF)
    sinT = np.ascontiguousarray(np.asarray(sin, dtype=np.float32).T).astype(BF)
    rotm = np.zeros((HD, HD), dtype=np.float32)
    for i in range(HD // 2):
        rotm[i + HD // 2, i] = -1.0   # rot[d'] = -q[d'+64] for d' < 64
        rotm[i, i + HD // 2] = 1.0    # rot[d'] = +q[d'-64] for d' >= 64
    rotm = rotm.astype(BF)
    oneh = np.zeros((128, 4 * QH), dtype=np.float32)
    for h in range(QH):
        oneh[:, 4 * h + h] = 1.0
    oneh = oneh.astype(BF)

    xT = [np.ascontiguousarray(hidden_states[b].T).astype(BF) for b in range(B)]
    in_maps = []
    for c in range(N_CORES):
        b, g = divmod(c, TPG)
        qr = w_qkv[512 * g:512 * (g + 1)]
        kr = w_qkv[HID + GATE + 256 * g:HID + GATE + 256 * (g + 1)]
        gr = w_qkv[HID + QH * g:HID + QH * (g + 1)]
        vr = w_qkv[HID + GATE + KV_DIM + 256 * g:HID + GATE + KV_DIM + 256 * (g + 1)]
        wqkvT = np.ascontiguousarray(
            np.concatenate([qr, kr, gr, vr], axis=0).T).astype(BF)
        woT = np.ascontiguousarray(w_o[:, 512 * g:512 * (g + 1)].T).astype(BF)
        in_maps.append({
            "xT": xT[b], "wqkvT": wqkvT, "woT": woT,
            "cosT": cosT, "sinT": sinT, "rotm": rotm, "oneh": oneh,
        })

    trace = bool(int(os.environ.get("KERNEL_TRACE", "0")))
    out = run_bass_kernel_spmd(nc, in_maps, list(range(N_CORES)), trace=trace)
    LAST_EXEC_NS = out.exec_time_ns
    LAST_RESULTS = out
    y = np.zeros((B, S, HID), dtype=np.float32)
    for c in range(N_CORES):
        b = c // TPG
        y[b] += out.results[c]["y"].astype(np.float32)
    return y
